# revision 11
# baseline (speedup 1.0000x reference)
"""Trainium2 (Bass/Tile) kernel for quantized multi-head attention.

Distributed across 8 NeuronCores: tensor-parallel over heads for the
Q4_0-dequant + QKV projections + RoPE + causal attention, one small
AllToAll per batch (overlapped with later batches), then a
token-parallel output projection. All weight transposes ride the DMA
X-bar (zero TensorE transposes). The full wo matrix is dequantized to
DRAM in 128-out-channel chunks interleaved through phase-1 DVE slack,
so the output projection is pure matmuls streaming panels from DRAM.
Host-side work is limited to input marshalling (sharding, layout
transposes of inputs, small derived tables) and stitching per-core
output token slices.
"""

import math
from dataclasses import dataclass

import numpy as np

import concourse.bass as bass
import concourse.tile as tile
from concourse import bacc, mybir, bass_isa

BF = mybir.dt.bfloat16
F32 = mybir.dt.float32
I8 = mybir.dt.int8
AOP = mybir.AluOpType
AF = mybir.ActivationFunctionType


@dataclass
class Cfg:
    B: int = 4
    S: int = 1024
    D: int = 4096
    NCORES: int = 8
    SCH: int = 512   # kept for test.py compat (unused)
    QCH: int = 512   # attention q-chunk

    @property
    def T(self):
        return self.B * self.S

    @property
    def H(self):
        return self.D // 128  # total heads (head_dim 128)

    @property
    def H_LOC(self):
        return self.H // self.NCORES

    @property
    def C_SHARD(self):
        return self.H_LOC * 128  # local channels

    @property
    def SPC(self):
        return self.S // self.NCORES  # seq slice per core per batch (128)

    @property
    def TPC(self):
        return self.B * self.SPC  # tokens per core (output slice)

    @property
    def NGP(self):
        return self.D // 128  # contraction k-tiles / group-pairs per row


def build_program(cfg: Cfg):
    """Build the per-core Bass program. Returns compiled nc."""
    c = cfg
    assert c.S % c.QCH == 0 and c.QCH <= 512
    assert c.S % (128 * c.NCORES) == 0

    # raise the stale SBUF cap (224KB phys, ~208 usable per partition)
    import concourse.tile_utils as tile_utils
    tile_utils.max_sbuf_usage = 208 * 1024

    nc = bacc.Bacc("TRN2", target_bir_lowering=False, debug=False,
                   num_devices=c.NCORES)

    OSH = c.C_SHARD  # qkv weight shard out-channels per core
    # ---- external I/O ----
    x_d = nc.dram_tensor("x", [c.D, c.T], BF, kind="ExternalInput")  # pre-transposed
    RPO = c.NGP          # packed rows per out-channel
    GPO = 2 * c.NGP      # scale groups per out-channel
    w_q = nc.dram_tensor("wq_w", [OSH * RPO, 64], I8, kind="ExternalInput")
    s_q = nc.dram_tensor("wq_s", [OSH * GPO, 1], BF, kind="ExternalInput")
    w_k = nc.dram_tensor("wk_w", [OSH * RPO, 64], I8, kind="ExternalInput")
    s_k = nc.dram_tensor("wk_s", [OSH * GPO, 1], BF, kind="ExternalInput")
    w_v = nc.dram_tensor("wv_w", [OSH * RPO, 64], I8, kind="ExternalInput")
    s_v = nc.dram_tensor("wv_s", [OSH * GPO, 1], BF, kind="ExternalInput")
    w_o = nc.dram_tensor("wo_w", [c.D * RPO, 64], I8, kind="ExternalInput")
    s_o = nc.dram_tensor("wo_s", [c.D * GPO, 1], BF, kind="ExternalInput")
    # rope tables, replicated over local heads; partition = s % 128
    cos4_d = nc.dram_tensor("cos4", [128, c.S // 128, c.C_SHARD], BF,
                            kind="ExternalInput")
    sins4_d = nc.dram_tensor("sins4", [128, c.S // 128, c.C_SHARD], BF,
                             kind="ExternalInput")
    maskd_d = nc.dram_tensor("maskd", [128, 128], BF, kind="ExternalInput")
    out_d = nc.dram_tensor("out", [c.TPC, c.D], BF, kind="ExternalOutput")

    # collective bounce buffers, one AllToAll per batch
    a2a_in = [nc.dram_tensor(f"a2a_in{b}", [c.NCORES, c.C_SHARD, c.SPC], BF)
              for b in range(c.B)]
    a2a_out = [nc.dram_tensor(f"a2a_out{b}", [c.NCORES, c.C_SHARD, c.SPC], BF)
               for b in range(c.B)]
    # dequantized+transposed full wo, staged via DRAM during phase 1
    wto_d = nc.dram_tensor("wto", [128, c.NGP, c.D], BF)

    inv_sqrt_d = 1.0 / math.sqrt(128.0)
    ngp = c.NGP
    half = ngp // 2  # 16 scale-group-pairs per xbar transpose slab

    def dequant_ob(sbuf, pw_v, ps_v, orow, write_comb):
        """Dequantize one 128-out-channel block; write_comb(g0, comb) sinks
        each [128, half, 128] bf16 slab (natural layout: partition = oc)."""
        p_nat = sbuf.tile([128, ngp * 64], I8, tag="dq_p", bufs=2)
        nc.sync.dma_start(p_nat[:], pw_v[orow:orow + 128, :])
        s_nat = sbuf.tile([128, ngp * 2], BF, tag="dq_s", bufs=2)
        nc.sync.dma_start(s_nat[:], ps_v[orow:orow + 128, :])
        for hb in range(2):
            g0 = hb * half
            comb = sbuf.tile([128, half, 128], BF, tag="dq_comb", bufs=2)
            # shift-free nibble extract: hi = b & 0xF0 == 16*msb
            # (scales table ships s_even/16 so the 16 cancels)
            msb = sbuf.tile([128, half * 64], I8, tag="dq_m", bufs=2)
            nc.vector.tensor_scalar(
                out=msb[:], in0=p_nat[:, g0 * 64:(g0 + half) * 64],
                scalar1=-16, scalar2=None, op0=AOP.bitwise_and)
            lsb = sbuf.tile([128, half * 64], I8, tag="dq_l", bufs=2)
            nc.vector.tensor_scalar(
                out=lsb[:], in0=p_nat[:, g0 * 64:(g0 + half) * 64],
                scalar1=15, scalar2=None, op0=AOP.bitwise_and)
            nc.vector.tensor_scalar(
                out=lsb[:], in0=lsb[:],
                scalar1=8, scalar2=None, op0=AOP.bitwise_xor)
            nc.vector.tensor_scalar(
                out=lsb[:], in0=lsb[:],
                scalar1=8, scalar2=None, op0=AOP.subtract)
            nc.vector.tensor_tensor(
                out=comb[:, :, 0:64],
                in0=msb[:].rearrange("o (gp f) -> o gp f", f=64),
                in1=s_nat[:, 2 * g0::2][:, :half, None].to_broadcast(
                    [128, half, 64]),
                op=AOP.mult)
            nc.vector.tensor_tensor(
                out=comb[:, :, 64:128],
                in0=lsb[:].rearrange("o (gp f) -> o gp f", f=64),
                in1=s_nat[:, 2 * g0 + 1::2][:, :half, None].to_broadcast(
                    [128, half, 64]),
                op=AOP.mult)
            write_comb(g0, comb)

    def view_wq(pw, ps):
        return (pw.ap().rearrange("(o r) f -> o (r f)", r=ngp),
                ps.ap().rearrange("(o g) one -> o (g one)", g=2 * ngp))

    def dequant_wt_ob(sbuf, wt, pw_v, ps_v, ob):
        def sink(g0, comb):
            nc.sync.dma_start(
                out=wt[:, g0:g0 + half, ob * 128:(ob + 1) * 128],
                in_=comb[:], transpose=True)
        dequant_ob(sbuf, pw_v, ps_v, ob * 128, sink)

    def dequant_dram_ob(sbuf, wt_dram, pw_v, ps_v, ob):
        """Transposed slab staged through a small SBUF tile into DRAM."""
        def sink(g0, comb):
            wtmp = sbuf.tile([128, half, 128], BF, tag="dq_wt", bufs=1)
            nc.sync.dma_start(out=wtmp[:], in_=comb[:], transpose=True)
            nc.sync.dma_start(
                out=wt_dram.ap()[:, g0:g0 + half, ob * 128:(ob + 1) * 128],
                in_=wtmp[:])
        dequant_ob(sbuf, pw_v, ps_v, ob * 128, sink)

    with tile.TileContext(nc) as tc:
        with tc.tile_pool(name="const", bufs=1) as const, \
             tc.tile_pool(name="sbuf", bufs=2) as sbuf:
            # constants
            cos4 = const.tile([128, c.S // 128, c.C_SHARD], BF)
            nc.sync.dma_start(cos4[:], cos4_d[:])
            sins4 = const.tile([128, c.S // 128, c.C_SHARD], BF)
            nc.sync.dma_start(sins4[:], sins4_d[:])
            maskd = const.tile([128, 128], BF)
            nc.sync.dma_start(maskd[:], maskd_d[:])

            # ============ phase 1: QKV + attention ============
            with tc.tile_pool(name="wt", bufs=1) as wtp, \
                 tc.tile_pool(name="xt", bufs=3) as xtp, \
                 tc.tile_pool(name="kqv", bufs=1) as kqvp, \
                 tc.tile_pool(name="pt", bufs=4) as ptp, \
                 tc.tile_pool(name="ppsum", bufs=3, space="PSUM") as ppsum, \
                 tc.tile_pool(name="spsum", bufs=3, space="PSUM") as spsum, \
                 tc.tile_pool(name="apsum", bufs=2, space="PSUM") as apsum:

                wt_q = wtp.tile([128, c.NGP, OSH], BF, tag="wt_q")
                wt_k = wtp.tile([128, c.NGP, OSH], BF, tag="wt_k")
                wt_v = wtp.tile([128, c.NGP, OSH], BF, tag="wt_v")
                q_v = view_wq(w_q, s_q)
                k_v = view_wq(w_k, s_k)
                v_v = view_wq(w_v, s_v)
                o_v = view_wq(w_o, s_o)
                # wq fully dequantized up front; wk/wv/wo interleaved below
                for ob in range(OSH // 128):
                    dequant_wt_ob(sbuf, wt_q, *q_v, ob)

                def project(b, ts, mat, wt_m, kt_b, qt_b, v_b):
                    tt0 = b * c.S + ts * 128
                    st0 = ts * 128
                    xt_ts = xtp.tile([128, c.NGP, 128], BF, tag="xt")
                    nc.sync.dma_start(
                        xt_ts[:],
                        x_d.ap().rearrange(
                            "(g p) t -> p g t", p=128)[:, :, tt0:tt0 + 128])
                    ps = ppsum.tile([128, OSH], F32, tag="proj")
                    for gp in range(c.NGP):
                        nc.tensor.matmul(
                            ps[:],
                            lhsT=xt_ts[:, gp, :],
                            rhs=wt_m[:, gp, :],
                            start=(gp == 0),
                            stop=(gp == c.NGP - 1))
                    if mat == "v":
                        nc.scalar.copy(out=v_b[:, ts, :], in_=ps[:])
                        return
                    # rope: roped = ps*cos4 + swaphalf(ps)*sins4
                    roped = sbuf.tile([128, c.C_SHARD], BF,
                                      tag="roped", bufs=3)
                    tmp = sbuf.tile([128, c.C_SHARD], BF,
                                    tag="ropetmp", bufs=3)
                    p3 = ps[:].rearrange("p (h d) -> p h d", d=128)
                    t3 = tmp[:].rearrange("p (h d) -> p h d", d=128)
                    c3 = cos4[:, ts, :].rearrange("p (h d) -> p h d", d=128)
                    s3 = sins4[:, ts, :].rearrange("p (h d) -> p h d", d=128)
                    nc.vector.tensor_tensor(
                        out=t3[:, :, 0:64], in0=p3[:, :, 64:128],
                        in1=s3[:, :, 0:64], op=AOP.mult)
                    nc.vector.tensor_tensor(
                        out=t3[:, :, 64:128], in0=p3[:, :, 0:64],
                        in1=s3[:, :, 64:128], op=AOP.mult)
                    nc.vector.tensor_tensor(
                        out=roped[:], in0=ps[:], in1=cos4[:, ts, :],
                        op=AOP.mult)
                    nc.vector.tensor_tensor(
                        out=roped[:], in0=roped[:], in1=tmp[:],
                        op=AOP.add)
                    dst = qt_b if mat == "q" else kt_b
                    # X-bar transpose per head: dst[d, h, st0+s] = roped[s, h*128+d]
                    nc.sync.dma_start(
                        out=dst[:, :, st0:st0 + 128],
                        in_=roped[:], transpose=True)

                def attention(b, kt_b, qt_b, v_b):
                    for h in range(c.H_LOC):
                        for qc in range(c.S // c.QCH):
                            q0 = qc * c.QCH
                            kmax = (q0 + c.QCH) // 128
                            at = apsum.tile([128, c.QCH], F32, tag="at")
                            psum_tree = sbuf.tile([128, c.QCH], F32,
                                                  tag="ptree", bufs=2)
                            for ki in range(kmax):
                                off = max(0, 128 * ki - q0)
                                stp = spsum.tile([128, c.QCH], F32, tag="sc")
                                nc.tensor.matmul(
                                    stp[:, off:],
                                    lhsT=kt_b[:, h, ki * 128:(ki + 1) * 128],
                                    rhs=qt_b[:, h, q0 + off:q0 + c.QCH],
                                    start=True, stop=True)
                                if 128 * ki >= q0:
                                    nc.vector.tensor_tensor(
                                        out=stp[:, off:off + 128],
                                        in0=stp[:, off:off + 128],
                                        in1=maskd[:], op=AOP.add)
                                pt = ptp.tile([128, c.QCH], BF, tag="pt")
                                nc.scalar.activation(
                                    out=pt[:, off:], in_=stp[:, off:],
                                    func=AF.Exp, scale=inv_sqrt_d)
                                # accumulate sum-over-k partials on DVE
                                if ki == 0:
                                    nc.vector.tensor_copy(
                                        out=psum_tree[:], in_=pt[:])
                                else:
                                    nc.vector.tensor_tensor(
                                        out=psum_tree[:, off:],
                                        in0=psum_tree[:, off:],
                                        in1=pt[:, off:], op=AOP.add)
                                nc.tensor.matmul(
                                    at[:, off:],
                                    lhsT=v_b[:, ki, h * 128:(h + 1) * 128],
                                    rhs=pt[:, off:],
                                    start=(ki == 0), stop=(ki == kmax - 1))
                            # z = sum over k-partitions, replicated to all
                            zfull = sbuf.tile([128, c.QCH], F32, tag="zf",
                                              bufs=2)
                            nc.gpsimd.partition_all_reduce(
                                zfull[:], psum_tree[:], channels=128,
                                reduce_op=bass_isa.ReduceOp.add)
                            rz = sbuf.tile([128, c.QCH], F32, tag="rz",
                                           bufs=2)
                            nc.vector.reciprocal_approx_fast(rz[:], zfull[:])
                            ao = sbuf.tile([128, c.QCH], BF, tag="ao")
                            nc.vector.tensor_tensor(
                                out=ao[:], in0=at[:], in1=rz[:], op=AOP.mult)
                            # scatter q-chunk to its 4 dest cores' seq slabs
                            nc.sync.dma_start(
                                out=a2a_in[b][qc * 4:(qc + 1) * 4,
                                              h * 128:(h + 1) * 128, :]
                                .rearrange("r c s -> c r s"),
                                in_=ao[:])
                    # per-batch collective, overlapped with later batches
                    nc.gpsimd.collective_compute(
                        "AllToAll", AOP.bypass,
                        replica_groups=[list(range(c.NCORES))],
                        ins=[a2a_in[b].ap().opt()],
                        outs=[a2a_out[b].ap().opt()],
                    )

                # ---- batch 0: mat-major so Q matmuls overlap wk/wv/wo
                # dequant; one extra weight block interleaved per ts ----
                kt_b = kqvp.tile([128, c.H_LOC, c.S], BF, tag="kt_b")
                qt_b = kqvp.tile([128, c.H_LOC, c.S], BF, tag="qt_b")
                v_b = kqvp.tile([128, c.S // 128, c.C_SHARD], BF, tag="v_b")
                for mat, wt_m in (("q", wt_q), ("k", wt_k), ("v", wt_v)):
                    for ts in range(c.S // 128):
                        project(0, ts, mat, wt_m, kt_b, qt_b, v_b)
                        if mat == "q" and ts % 2 == 0:
                            dequant_wt_ob(sbuf, wt_k, *k_v, ts // 2)
                        elif mat == "k" and ts % 2 == 0:
                            dequant_wt_ob(sbuf, wt_v, *v_v, ts // 2)
                        elif mat == "v":
                            dequant_dram_ob(sbuf, wto_d, *o_v, ts)
                attention(0, kt_b, qt_b, v_b)

                # ---- batches 1-3: one wo block interleaved per ts ----
                for b in range(1, c.B):
                    kt_b = kqvp.tile([128, c.H_LOC, c.S], BF, tag="kt_b")
                    qt_b = kqvp.tile([128, c.H_LOC, c.S], BF, tag="qt_b")
                    v_b = kqvp.tile([128, c.S // 128, c.C_SHARD], BF,
                                    tag="v_b")
                    for ts in range(c.S // 128):
                        for mat, wt_m in (("q", wt_q), ("k", wt_k),
                                          ("v", wt_v)):
                            project(b, ts, mat, wt_m, kt_b, qt_b, v_b)
                        dequant_dram_ob(sbuf, wto_d, *o_v,
                                        8 * b + ts)
                    attention(b, kt_b, qt_b, v_b)

            # ==== phase 2: output projection (token-sharded, pure MMs) ====
            with tc.tile_pool(name="gath", bufs=1) as gathp, \
                 tc.tile_pool(name="wop", bufs=2) as wopp, \
                 tc.tile_pool(name="wpsum", bufs=2, space="PSUM") as wpsum:
                gath = gathp.tile([128, c.NGP, c.TPC], BF)
                for b in range(c.B):
                    nc.sync.dma_start(
                        gath[:, :, b * c.SPC:(b + 1) * c.SPC],
                        a2a_out[b].ap().rearrange("s (g p) t -> p (s g) t",
                                                  p=128))
                for oc in range(c.D // 512):
                    panel = wopp.tile([128, c.NGP, 512], BF, tag="wop")
                    nc.sync.dma_start(
                        panel[:], wto_d.ap()[:, :, oc * 512:(oc + 1) * 512])
                    for tb in range(c.TPC // 128):
                        ops = wpsum.tile([128, 512], F32, tag="wo")
                        for ct in range(c.NGP):
                            nc.tensor.matmul(
                                ops[:], lhsT=gath[:, ct, tb * 128:(tb + 1) * 128],
                                rhs=panel[:, ct, :],
                                start=(ct == 0), stop=(ct == c.NGP - 1))
                        osb = sbuf.tile([128, 512], BF, tag="osb", bufs=3)
                        nc.scalar.copy(out=osb[:], in_=ops[:])
                        nc.sync.dma_start(
                            out=out_d[tb * 128:(tb + 1) * 128,
                                      oc * 512:(oc + 1) * 512],
                            in_=osb[:])

    nc.compile()
    return nc


# ---------------- host-side input prep ----------------

def prep_core_inputs(cfg: Cfg, x, cos_half, sin_half, mask,
                     wq_w, wq_s, wk_w, wk_s, wv_w, wv_s, wo_w, wo_s):
    """Build in_maps (list of dicts, one per core) from full inputs."""
    import ml_dtypes
    c = cfg
    bf16 = ml_dtypes.bfloat16
    HD2 = 64

    x2 = np.ascontiguousarray(
        np.asarray(x).reshape(c.T, c.D).T)  # ship transposed [D, T]

    # rope tables [128, S//128, C_SHARD]
    ch = np.asarray(cos_half, np.float32)  # [S, 64]
    sh = np.asarray(sin_half, np.float32)
    cos = np.concatenate([ch, ch], axis=1).astype(bf16).astype(np.float32)  # [S,128]
    sin = np.concatenate([sh, sh], axis=1).astype(bf16).astype(np.float32)
    sins = sin.copy()
    sins[:, :HD2] = -sin[:, :HD2]
    cos4 = np.tile(cos[:, None, :], (1, c.H_LOC, 1)).reshape(c.S, c.C_SHARD)
    sins4 = np.tile(sins[:, None, :], (1, c.H_LOC, 1)).reshape(c.S, c.C_SHARD)
    # partition = s % 128, ssub = s // 128
    cos4 = np.ascontiguousarray(
        cos4.reshape(c.S // 128, 128, c.C_SHARD).transpose(1, 0, 2)).astype(bf16)
    sins4 = np.ascontiguousarray(
        sins4.reshape(c.S // 128, 128, c.C_SHARD).transpose(1, 0, 2)).astype(bf16)

    # diagonal mask block: maskd[k, q] from input mask[q, k] (first 128 block)
    m = np.asarray(mask, np.float32)[:128, :128]
    maskd = np.maximum(m.T, -1e30).astype(bf16)

    OSH = c.C_SHARD

    def dq_scales(ps):
        # [N*GPO, 1] -> even groups (msb) divided by 16 (exact in bf16)
        a = np.asarray(ps).astype(np.float32).reshape(-1, 2)
        a[:, 0] /= 16.0
        return np.ascontiguousarray(a.reshape(-1, 1)).astype(bf16)

    in_maps = []
    for core in range(c.NCORES):
        RPO = c.NGP
        r0 = core * OSH * RPO
        g0 = core * OSH * 2 * RPO
        in_maps.append({
            "x": x2.astype(bf16, copy=False),
            "wq_w": np.ascontiguousarray(np.asarray(wq_w)[r0:r0 + OSH * RPO]),
            "wq_s": dq_scales(np.asarray(wq_s)[g0:g0 + OSH * 2 * RPO]),
            "wk_w": np.ascontiguousarray(np.asarray(wk_w)[r0:r0 + OSH * RPO]),
            "wk_s": dq_scales(np.asarray(wk_s)[g0:g0 + OSH * 2 * RPO]),
            "wv_w": np.ascontiguousarray(np.asarray(wv_w)[r0:r0 + OSH * RPO]),
            "wv_s": dq_scales(np.asarray(wv_s)[g0:g0 + OSH * 2 * RPO]),
            "wo_w": np.ascontiguousarray(np.asarray(wo_w)),
            "wo_s": dq_scales(wo_s),
            "cos4": cos4,
            "sins4": sins4,
            "maskd": maskd,
        })
    return in_maps


def unshard_output(cfg: Cfg, results):
    """results: list per core of {"out": [TPC, D]}; core r's rows are
    (b, s1) with seq slice [128r, 128(r+1)) of every batch."""
    c = cfg
    full = np.empty((c.B, c.S, c.D), dtype=np.asarray(results[0]["out"]).dtype)
    for r in range(c.NCORES):
        o = np.asarray(results[r]["out"]).reshape(c.B, c.SPC, c.D)
        full[:, r * c.SPC:(r + 1) * c.SPC, :] = o
    return full


# ======================================================================
# Self-contained kernel entry point.
# Accepts FULL (unsharded) inputs as produced by setup_inputs() and
# returns the FULL output [B, S, D] (bfloat16), matching reference().
# ======================================================================

_CACHE = {}


def _get_program(cfg):
    key = (cfg.B, cfg.S, cfg.D, cfg.NCORES, cfg.SCH, cfg.QCH)
    if key not in _CACHE:
        _CACHE[key] = build_program(cfg)
    return _CACHE[key]


def kernel(x, start_pos=0, cos_half=None, sin_half=None, mask=None,
           wq_w=None, wq_s=None, wk_w=None, wk_s=None,
           wv_w=None, wv_s=None, wo_w=None, wo_s=None,
           cache_k_w=None, cache_k_s=None, cache_v_w=None, cache_v_s=None,
           **_unused):
    from concourse.bass_utils import run_bass_kernel_spmd

    assert int(start_pos) == 0, "kernel specialised for start_pos == 0"
    x = np.asarray(x)
    B, S, D = x.shape
    cfg = Cfg(B=B, S=S, D=D, NCORES=8, SCH=512, QCH=512)
    # start_pos==0 with S==MAX_S, B==MAX_B: the quantized KV cache is fully
    # overwritten before use, so cache_* inputs cannot affect the output.
    in_maps = prep_core_inputs(cfg, x, cos_half, sin_half, mask,
                               wq_w, wq_s, wk_w, wk_s, wv_w, wv_s,
                               wo_w, wo_s)
    nc = _get_program(cfg)
    res = run_bass_kernel_spmd(nc, in_maps, core_ids=list(range(cfg.NCORES)))
    out = unshard_output(cfg, res.results)
    import ml_dtypes
    return out.astype(ml_dtypes.bfloat16, copy=False)
